# revision 6
# baseline (speedup 1.0000x reference)
"""GCNConv (PyG-faithful, normalize=True, add_self_loops=True) on 8 Trainium2
NeuronCores via Bass/Tile.

Strategy (1D graph/data parallel, dst-sharded):
  - Nodes partitioned across 8 cores (12500 rows each, padded to 12544).
  - Phase A: each core computes g = dinv*(x @ W) in 4 sub-shards; each
    sub-shard is AllGathered independently so phase B can start early.
    g_table DRAM layout is chunk-major: chunk j = all cores' sub-shard j
    (<=25600 rows, so gather idxs fit int16).
  - Phase B: edges (self-loops excluded) are host-sorted into 56 sections
    (14 windows x 4 chunks); within a section, rows are laid out in per-block
    spans sized max-over-cores (SPMD-shared layout, ~10% pad). One SWDGE
    dma_gather per section fetches g rows (bf16) into SBUF. Per tile, a
    [128,256] two-block one-hot is built on VectorE (is_equal vs a host
    iota256 constant, keyed by per-row dl values); TensorE segment-sums each
    tile into per-block PSUM accumulators (1-2 matmuls per tile). The added
    self-loop is one identity matmul per block on the core's own g rows.
    Epilogue scales by dinv_dst (ScalarE), adds bias (VectorE), stores.
"""

import sys

if "/opt/trn_rl_repo" not in sys.path:
    sys.path.insert(0, "/opt/trn_rl_repo")

import numpy as np

P = 128
NCORES = 8
WBLK = 7                 # blocks per window
NWIN = 14                # windows per core (98 blocks)
CHUNKS = 4
_PAD_DL = 300.0          # sentinel dl -> all-zero sel column


def _pack(x, edge_index, weight, b):
    """Host-side index/structure preprocessing (no numeric compute on x/W)."""
    import ml_dtypes

    x = np.ascontiguousarray(np.asarray(x, dtype=np.float32))
    ei = np.asarray(edge_index)
    weight = np.ascontiguousarray(np.asarray(weight, dtype=np.float32))
    bias = np.asarray(b, dtype=np.float32).reshape(-1)

    n, nin = x.shape
    assert nin == P and weight.shape == (P, P)
    nb = n // NCORES                       # 12500
    blocks = (nb + P - 1) // P             # 98
    nbp = blocks * P                       # 12544
    assert blocks == NWIN * WBLK

    # sub-shard split in tiles: [25, 25, 24, 24]
    sub_t = [25, 25, 24, 24]
    sz = [t * P for t in sub_t]            # rows per sub-shard
    off = np.cumsum([0] + sz)[:-1]         # [0, 3200, 6400, 9472]
    chunk_rows = [NCORES * s for s in sz]  # [25600, 25600, 24576, 24576]
    cbase = np.cumsum([0] + chunk_rows)[:-1]

    src = ei[0].astype(np.int64)
    dst = ei[1].astype(np.int64)
    m = src.shape[0]

    deg = np.bincount(dst, minlength=n).astype(np.float32) + 1.0
    dinv = 1.0 / np.sqrt(deg)

    # edge -> section (dst core k, window w, src chunk j) and block-in-window
    k = dst // nb
    dlc = dst - k * nb
    w = dlc // (WBLK * P)
    bw = (dlc % (WBLK * P)) // P           # block in window 0..6
    dl_local = dlc % P

    ks = src // nb
    l = src - ks * nb
    j = (l >= off[1]).astype(np.int64) + (l >= off[2]) + (l >= off[3])
    szs = np.array(sz)[j]
    offs = np.array(off)[j]
    row_in_chunk = ks * szs + (l - offs)   # gather idx within chunk j

    # counts per (core, window, chunk, block)
    cnt = np.zeros((NCORES, NWIN, CHUNKS, WBLK), np.int64)
    np.add.at(cnt, (k, w, j, bw), 1)
    R = cnt.max(axis=0)                    # [NWIN, CHUNKS, WBLK] shared spans

    # shared section layout
    span_start = np.zeros((NWIN, CHUNKS, WBLK), np.int64)
    T_sec = np.zeros((NWIN, CHUNKS), np.int64)   # tiles per section
    sec_tilebase = np.zeros((NWIN, CHUNKS), np.int64)
    t_total = 0
    for wi in range(NWIN):
        for ji in range(CHUNKS):
            rows = 0
            for bi in range(WBLK):
                span_start[wi, ji, bi] = rows
                rows += int(R[wi, ji, bi])
            T = (rows + P - 1) // P
            T_sec[wi, ji] = T
            sec_tilebase[wi, ji] = t_total
            t_total += T

    # per-tile matmul schedule (shared): which blocks intersect each tile
    # btile[t] = first block whose span intersects tile t (the iota base blk)
    tile_first_blk = np.zeros(t_total, np.int64)
    sched = []  # per (w): list of (tile_global, phase, bw, start, stop)
    for wi in range(NWIN):
        per_tile_blocks = {}
        first_last = {}
        for ji in range(CHUNKS):
            tb = int(sec_tilebase[wi, ji])
            for bi in range(WBLK):
                s = int(span_start[wi, ji, bi])
                e = s + int(R[wi, ji, bi])
                if e == s:
                    continue
                t0, t1 = s // P, (e - 1) // P
                for t in range(tb + t0, tb + t1 + 1):
                    per_tile_blocks.setdefault(t, []).append(bi)
        for t, bis in per_tile_blocks.items():
            tile_first_blk[t] = min(bis)
        # per block: ordered tile list across chunks
        wsched = []
        for t in sorted(per_tile_blocks):
            for bi in sorted(per_tile_blocks[t]):
                wsched.append([t, bi])
        # mark first/last per block
        seen = {}
        for idx_, (t, bi) in enumerate(wsched):
            seen.setdefault(bi, []).append(idx_)
        entries = []
        remaining = {bi: len(seen[bi]) for bi in seen}
        for idx_, (t, bi) in enumerate(wsched):
            ph = bi - int(tile_first_blk[t])
            assert 0 <= ph <= 1, (wi, t, bi, ph)
            last = idx_ == seen[bi][-1]
            remaining[bi] -= 1
            # pair is (bi//2*2, bi//2*2+1); epilogue when both drained
            pi = bi // 2
            mates = [b for b in (2 * pi, 2 * pi + 1) if b < WBLK]
            pair_done = last and all(remaining.get(b, 0) == 0 for b in mates)
            entries.append((t, ph, bi, last, pair_done))
        sched.append(entries)

    # per-core row placement
    order = np.lexsort((src, bw, j, w, k))  # stable by (k, w, j, bw)
    ks_o, ws_o, js_o, bs_o = k[order], w[order], j[order], bw[order]
    ric_o = row_in_chunk[order].astype(np.int32)
    dl_o = dl_local[order].astype(np.int32)
    # rank within (k,w,j,b) group
    gkey = ((ks_o * NWIN + ws_o) * CHUNKS + js_o) * WBLK + bs_o
    gstart = np.zeros(NCORES * NWIN * CHUNKS * WBLK, np.int64)
    gc = np.bincount(gkey, minlength=gstart.shape[0])
    gstart[1:] = np.cumsum(gc)[:-1]
    rank = np.arange(m) - gstart[gkey]

    dest = (
        sec_tilebase[ws_o, js_o] * P
        + span_start[ws_o, js_o, bs_o]
        + rank
    )
    # dl relative to the tile's first block
    tg = sec_tilebase[ws_o, js_o] + (span_start[ws_o, js_o, bs_o] + rank) // P
    dl_rel = dl_o + P * (bs_o - tile_first_blk[tg])

    idx_lin = np.zeros((NCORES, t_total * P), np.int16)
    dl_lin = np.full((NCORES, t_total * P), _PAD_DL, np.float32)
    idx_lin[ks_o, dest] = ric_o.astype(np.int16)
    dl_lin[ks_o, dest] = dl_rel

    # wrap-16 + replicate to 128 partitions (dma_gather idx layout)
    l16 = t_total * P // 16
    idx_w = idx_lin.reshape(NCORES, l16, 16).transpose(0, 2, 1)
    idx_pack = np.ascontiguousarray(np.tile(idx_w, (1, NCORES, 1)))  # [8,128,L16]

    # dl as [128 rows-in-tile, t_total] bf16
    dl_pack = np.ascontiguousarray(
        dl_lin.reshape(NCORES, t_total, P).transpose(0, 2, 1)
    ).astype(np.float32)

    # iota256 [128, 256] (rows identical), identity [128,128], both bf16
    iota256 = np.tile(np.arange(256, dtype=np.float32)[None, :], (P, 1)).astype(
        ml_dtypes.bfloat16
    )
    ident = np.eye(P, dtype=np.float32).astype(ml_dtypes.bfloat16)

    # per-core xT, dinv
    xt = np.zeros((NCORES, P, nbp), np.float32)
    dinv_t = np.zeros((NCORES, P, blocks), np.float32)
    for kk in range(NCORES):
        xs = x[kk * nb : (kk + 1) * nb]
        xt[kk, :, :nb] = xs.T
        dv = np.zeros(nbp, np.float32)
        dv[:nb] = dinv[kk * nb : (kk + 1) * nb]
        dinv_t[kk] = dv.reshape(blocks, P).T
    bias_rep = np.ascontiguousarray(np.tile(bias[None, :], (P, 1)))

    meta = dict(
        n=n, nb=nb, blocks=blocks, nbp=nbp,
        sub_t=sub_t, sz=sz, off=list(off), chunk_rows=chunk_rows,
        cbase=list(cbase), T_sec=T_sec, sec_tilebase=sec_tilebase,
        t_total=t_total, l16=l16, sched=sched,
    )
    in_maps = [
        {
            "xt": xt[kk],
            "w_in": weight,
            "bias": bias_rep,
            "dinv": dinv_t[kk],
            "idxp": idx_pack[kk],
            "dlp": dl_pack[kk],
            "iota": iota256,
            "ident": ident,
        }
        for kk in range(NCORES)
    ]
    return meta, in_maps


def _build_program(meta):
    from concourse import bass, bacc, mybir
    import concourse.tile as tile

    blocks = meta["blocks"]
    nbp = meta["nbp"]
    sub_t = meta["sub_t"]
    sz = meta["sz"]
    off = meta["off"]
    chunk_rows = meta["chunk_rows"]
    cbase = meta["cbase"]
    T_sec = meta["T_sec"]
    sec_tilebase = meta["sec_tilebase"]
    t_total = meta["t_total"]
    l16 = meta["l16"]
    sched = meta["sched"]
    npad = sum(chunk_rows)
    secmax = int(T_sec.max())

    f32 = mybir.dt.float32
    bf16 = mybir.dt.bfloat16

    nc = bacc.Bacc(num_swdge_queues=4)
    xt_in = nc.declare_dram_parameter("xt", [P, nbp], f32, isOutput=False)
    w_in = nc.declare_dram_parameter("w_in", [P, P], f32, isOutput=False)
    bias_in = nc.declare_dram_parameter("bias", [P, P], f32, isOutput=False)
    dinv_in = nc.declare_dram_parameter("dinv", [P, blocks], f32, isOutput=False)
    idx_in = nc.declare_dram_parameter("idxp", [P, l16], mybir.dt.int16, isOutput=False)
    dl_in = nc.declare_dram_parameter("dlp", [P, t_total], f32, isOutput=False)
    iota_in = nc.declare_dram_parameter("iota", [P, 2 * P], bf16, isOutput=False)
    id_in = nc.declare_dram_parameter("ident", [P, P], bf16, isOutput=False)
    out_ext = nc.declare_dram_parameter("out", [nbp, P], f32, isOutput=True)

    h_shard = nc.dram_tensor("h_shard", [nbp, P], bf16)
    g_table = nc.dram_tensor("g_table", [npad, P], bf16, addr_space="Shared")

    with tile.TileContext(nc) as tc:
        with (
            tc.tile_pool(name="const", bufs=1) as cpool,
            tc.tile_pool(name="pha", bufs=2) as apool,
            tc.tile_pool(name="msgp", bufs=2) as mpool,
            tc.tile_pool(name="selp", bufs=8) as spool,
            tc.tile_pool(name="ownp", bufs=2) as gpool,
            tc.tile_pool(name="outp", bufs=2) as opool,
            tc.tile_pool(name="psA", bufs=2, space="PSUM") as psA,
            tc.tile_pool(name="psB", bufs=1, space="PSUM") as psB,
        ):
            # constants
            w_sb = cpool.tile([P, P], f32, tag="w")
            nc.sync.dma_start(out=w_sb[:], in_=w_in[:])
            bias_sb = cpool.tile([P, P], f32, tag="bias")
            nc.sync.dma_start(out=bias_sb[:], in_=bias_in[:])
            dinv_sb = cpool.tile([P, blocks], f32, tag="dinv")
            nc.sync.dma_start(out=dinv_sb[:], in_=dinv_in[:])
            iota_sb = cpool.tile([P, 2 * P], bf16, tag="iota")
            nc.sync.dma_start(out=iota_sb[:], in_=iota_in[:])
            id_sb = cpool.tile([P, P], bf16, tag="ident")
            nc.sync.dma_start(out=id_sb[:], in_=id_in[:])
            dl_sb = cpool.tile([P, t_total], f32, tag="dl")
            nc.sync.dma_start(out=dl_sb[:], in_=dl_in[:])
            idx_sb = cpool.tile([P, l16], mybir.dt.int16, tag="idx")
            for i in range(4):
                s = l16 // 4
                e = l16 if i == 3 else (i + 1) * s
                nc.sync.dma_start(out=idx_sb[:, i * s : e], in_=idx_in[:, i * s : e])

            # ---- phase A: per sub-shard, h = dinv*(x @ W) -> AllGather_j
            for jph in range(CHUNKS):
                tj = sub_t[jph]
                xt_t = apool.tile([P, 25 * P], f32, tag="xt")
                nc.sync.dma_start(
                    out=xt_t[:, : tj * P],
                    in_=xt_in[:, off[jph] : off[jph] + sz[jph]],
                )
                hbig = apool.tile([P, 25, P], bf16, tag="hbig")
                for t in range(tj):
                    ph = psA.tile([P, P], f32, tag="ph")
                    nc.tensor.matmul(
                        out=ph[:],
                        lhsT=xt_t[:, t * P : (t + 1) * P],
                        rhs=w_sb[:],
                        start=True,
                        stop=True,
                    )
                    gb = off[jph] // P + t
                    nc.vector.tensor_scalar(
                        out=hbig[:, t, :],
                        in0=ph[:],
                        scalar1=dinv_sb[:, gb : gb + 1],
                        scalar2=None,
                        op0=mybir.AluOpType.mult,
                    )
                nc.sync.dma_start(
                    out=h_shard[off[jph] : off[jph] + sz[jph], :].rearrange(
                        "(t p) f -> p t f", p=P
                    ),
                    in_=hbig[:, :tj, :],
                )
                nc.gpsimd.collective_compute(
                    "AllGather",
                    mybir.AluOpType.bypass,
                    replica_groups=[list(range(NCORES))],
                    ins=[h_shard[off[jph] : off[jph] + sz[jph], :]],
                    outs=[g_table[cbase[jph] : cbase[jph] + chunk_rows[jph], :]],
                )

            # ---- phase B
            for wi in range(NWIN):
                # gathers: one per (window, chunk) section
                msgs = [None] * CHUNKS
                for ji in range(CHUNKS):
                    T = int(T_sec[wi, ji])
                    sec0 = int(sec_tilebase[wi, ji])
                    msg = mpool.tile([P, secmax, P], bf16, tag=f"msg{ji}")
                    nc.gpsimd.dma_gather(
                        out_ap=msg[:, :T, :],
                        in_ap=g_table[cbase[ji] : cbase[ji] + chunk_rows[ji], :],
                        idxs_ap=idx_sb[:, sec0 * 8 : (sec0 + T) * 8],
                        num_idxs=T * P,
                        num_idxs_reg=T * P,
                        elem_size=P,
                        single_packet=False,
                        queue_num=ji,
                    )
                    msgs[ji] = (msg, sec0)

                # own-shard g rows for self-loop matmuls
                gown = gpool.tile([P, WBLK, P], bf16, tag="gown")
                nc.sync.dma_start(
                    out=gown[:],
                    in_=h_shard[wi * WBLK * P : (wi + 1) * WBLK * P, :].rearrange(
                        "(t p) f -> p t f", p=P
                    ),
                )

                accds = []
                for pi in range(4):
                    acc_t = psB.tile([P, 2, P], f32, tag=f"accd{pi}", name=f"accd{pi}")
                    accds.append(acc_t)
                osb_w = opool.tile([P, WBLK, P], f32, tag="osbw")

                # one identity matmul per pair opens both halves (self-loops)
                for pi in range(4):
                    nw = 2 if 2 * pi + 1 < WBLK else 1
                    nc.tensor.matmul(
                        out=accds[pi][:, :nw, :],
                        lhsT=id_sb[:],
                        rhs=gown[:, 2 * pi : 2 * pi + nw, :],
                        start=True,
                        stop=False,
                    )

                # tile-major scatter matmuls
                sel_cache = {}
                for (t, phs, bi, last, pair_done) in sched[wi]:
                    # find the section owning tile t
                    ji = 0
                    for jq in range(CHUNKS):
                        if sec_tilebase[wi, jq] <= t < sec_tilebase[wi, jq] + T_sec[wi, jq]:
                            ji = jq
                            break
                    msg, sec0 = msgs[ji]
                    tloc = t - sec0
                    if t not in sel_cache:
                        sel = spool.tile([P, 2 * P], bf16, tag="sel")
                        nc.vector.tensor_scalar(
                            out=sel[:],
                            in0=iota_sb[:],
                            scalar1=dl_sb[:, t : t + 1],
                            scalar2=None,
                            op0=mybir.AluOpType.is_equal,
                        )
                        sel_cache[t] = sel
                    sel = sel_cache[t]
                    nc.tensor.matmul(
                        out=accds[bi // 2][:, bi % 2, :],
                        lhsT=sel[:, phs * P : (phs + 1) * P],
                        rhs=msg[:, tloc, :],
                        start=False,
                        stop=last,
                    )
                    if pair_done:
                        pi = bi // 2
                        for bq in (2 * pi, 2 * pi + 1):
                            if bq >= WBLK:
                                continue
                            nc.scalar.activation(
                                out=osb_w[:, bq, :],
                                in_=accds[pi][:, bq % 2, :],
                                func=mybir.ActivationFunctionType.Copy,
                                scale=dinv_sb[:, wi * WBLK + bq : wi * WBLK + bq + 1],
                            )

                nc.vector.tensor_tensor(
                    out=osb_w[:],
                    in0=osb_w[:],
                    in1=bias_sb[:].unsqueeze(1).to_broadcast([P, WBLK, P]),
                    op=mybir.AluOpType.add,
                )
                nc.sync.dma_start(
                    out=out_ext[wi * WBLK * P : (wi + 1) * WBLK * P, :].rearrange(
                        "(j p) f -> p j f", p=P
                    ),
                    in_=osb_w[:],
                )

    nc.finalize()
    return nc


def _run(inputs, trace=False, trace_cores=None):
    from concourse.bass_utils import run_bass_kernel_spmd

    meta, in_maps = _pack(**inputs)
    nc = _build_program(meta)
    res = run_bass_kernel_spmd(
        nc,
        in_maps,
        list(range(NCORES)),
        trace=trace,
        trace_cores=trace_cores,
    )
    n, nb = meta["n"], meta["nb"]
    out = np.empty((n, P), np.float32)
    for kk in range(NCORES):
        out[kk * nb : (kk + 1) * nb] = np.asarray(res.results[kk]["out"])[:nb]
    return out, res


def kernel(x, edge_index, weight, b):
    out, _ = _run(dict(x=x, edge_index=edge_index, weight=weight, b=b))
    return out


if __name__ == "__main__":
    rng = np.random.default_rng(0)
    n, e = 100000, 1600000
    x = rng.standard_normal((n, P), dtype=np.float32)
    ei = rng.integers(0, n, (2, e)).astype(np.int64)
    w = (rng.standard_normal((P, P)) / np.sqrt(P)).astype(np.float32)
    bb = (rng.standard_normal(P) * 0.02).astype(np.float32)
    out = kernel(x, ei, w, bb)
    print("out", out.shape, out.dtype)
